# revision 9
# baseline (speedup 1.0000x reference)
"""Trainium2 Bass kernel for nn_CkConv1D (continuous-kernel causal conv).

Math: the reference materializes a T x T Toeplitz kernel
K[o,c,i,j] = sum_h w2[h]*sin(A_h*(j-i) + off[o,c,h]) + b2  (A_h = w1[h,0]/T),
masks it causally (j<=i) and contracts with x.  With
phi = off - A_h*i and sin(A_h*j + phi) = sin(A_h j)cos(phi) + cos(A_h j)sin(phi),
the masked contraction reduces to causal prefix sums
  S_s[i,c,h] = sum_{j<=i} sin(A_h j) x[j,c],   S_c likewise with cos.
Splitting phi = U + V with U = off[o,c,h] (i-independent) and V = -A_h*i:
  y[i,o] = sum_p cosU[p,o]*Z1[p,i] + sinU[p,o]*Z2[p,i] + b2*sum_c Sx[i,c]
  Z1 = cos(Ai)*S's - sin(Ai)*S'c,  Z2 = cos(Ai)*S'c + sin(Ai)*S's
(p = (c,h) packed on 128 partitions, w2 folded into S').

Work is sharded over 8 NeuronCores: core m produces output rows
[128m, 128m+128).  Per core, S' = (within-window prefix via one
upper-triangular matmul) + (carry-in over earlier 128-blocks via two
[128x128]x[128x32] matmuls and a masked reduction).  All trig tables are
weight/position-derived and precomputed on the host (RoPE-style caches);
only x-dependent math runs on device.  Everything is bf16 into fp32 PSUM.

The program is identical on every core (SPMD); per-core behavior comes only
from per-core input data (its x window, causally masked x, per-core tables).
"""

import sys
from pathlib import Path

import numpy as np

for _p in ("/opt/trn_rl_repo",):
    if _p not in sys.path and Path(_p).exists():
        sys.path.insert(0, _p)

import ml_dtypes

import concourse.bass as bass  # noqa: F401
import concourse.bacc as bacc
import concourse.tile as tile
from concourse import mybir
from concourse.bass_utils import run_bass_kernel_spmd

# Constrain walrus's semaphore allocation (default max-sem-num=150).  The
# NEFF epilogue zeroes the semaphore file with one instruction per
# semaphore split across engines; a tighter allocation lets the compiler
# overlap/schedule that teardown better (~2.5us on this kernel).  Bass's
# kernel-semaphore pool must sit above the walrus range.
_MAX_SEM = 80
import concourse.bass as _bm
import concourse.bass_utils as _bu

_bm.get_kernel_semaphore_range = lambda: range(_MAX_SEM, 256)
_orig_bvo = _bu.bir_verify_and_optimise


def _bvo(tmpdir, inp="bir.json", outp="file.neff", arch=None, *, dve_root=None):
    _saved = _bu.get_walrus_args

    def _wa(*a, **k):
        return _saved(*a, **k) + [f"--max-sem-num={_MAX_SEM}"]

    _bu.get_walrus_args = _wa
    try:
        return _orig_bvo(tmpdir, inp, outp, arch, dve_root=dve_root)
    finally:
        _bu.get_walrus_args = _saved


_bu.bir_verify_and_optimise = _bvo

F32 = mybir.dt.float32
BF16 = mybir.dt.bfloat16
BF = ml_dtypes.bfloat16
T, C, O, H, P, M = 1024, 4, 2, 32, 128, 8

# tA columns (tensor-engine inputs, gpsimd DMA queue)
A_COSJ = 0        # cos(A_h*jj) tiled over c            [128, 128]
A_SINJ = 128      # sin(A_h*jj) tiled over c            [128, 128]
A_XM = 256        # masked x, cols (b,c)                [128, 32]
A_ONES4 = 288     # ones                                [128, 4]
A_UUC = 292       # cos(off[o,c,h]), cols o             [128, 2]
A_UUS = 294       # sin(off[o,c,h])                     [128, 2]
A_BB = 296        # rows 0:4 = b2                       [4, 2]
NA = 298

# tB columns (vector-engine inputs, scalar DMA queue)
B_TWSC = 0        # w2[h]*sin|cos(A_h*(128m+jj)), (t,c,h) [128, 256]
B_XWREP = 256     # x window replicated to (t,c,h)        [128, 256]
B_XWIN = 512      # x window, cols c                      [128, 4]
NB = 516

# tC: ut upper-triangular ones [128, 128] (sync DMA queue)

# tD columns (sync DMA queue)
D_CVI = 0         # cos(A_h*(128m+ii)), rows (c,h)      [128, 128]
D_SVI = 128       # sin(A_h*(128m+ii))                  [128, 128]
D_MA = 256        # [Ms | Mc] carry mask                [128, 64]
D_MB = 320        # [Mc | -Ms]                          [128, 64]
D_MX = 384        # rows 0:4: delta_{c,c'}              [4, 32]
ND = 416

_nc_cache = {}


def _build_nc():
    nc = bacc.Bacc()
    ta = nc.dram_tensor("ta", [P, NA], BF16, kind="ExternalInput")
    tb = nc.dram_tensor("tb", [P, NB], BF16, kind="ExternalInput")
    tcu = nc.dram_tensor("tc", [P, P], BF16, kind="ExternalInput")
    td = nc.dram_tensor("td", [P, ND], BF16, kind="ExternalInput")
    y = nc.dram_tensor("y", [O, P], F32, kind="ExternalOutput")

    Ident = mybir.ActivationFunctionType.Identity
    Mult = mybir.AluOpType.mult
    Add = mybir.AluOpType.add

    with tile.TileContext(nc) as tc:
        with (
            tc.tile_pool(name="sb", bufs=1) as sb,
            tc.tile_pool(name="ps", bufs=1, space="PSUM") as ps,
        ):
            ta_sb = sb.tile([P, NA], BF16)
            tb_sb = sb.tile([P, NB], BF16)
            tc_sb = sb.tile([P, P], BF16)
            td_sb = sb.tile([P, ND], BF16)
            # one input DMA per DMA-capable queue (gpsimd/scalar/sync) so
            # the ~800ns issue cost overlaps instead of serializing
            nc.gpsimd.dma_start(out=ta_sb[:], in_=ta[:])
            nc.scalar.dma_start(out=tb_sb[:], in_=tb[:])
            nc.sync.dma_start(out=tc_sb[:], in_=tcu[:])
            nc.sync.dma_start(out=td_sb[:], in_=td[:])

            cosJb = ta_sb[:, A_COSJ:A_COSJ + P]
            sinJb = ta_sb[:, A_SINJ:A_SINJ + P]
            xm = ta_sb[:, A_XM:A_XM + M * C]
            ones4 = ta_sb[:, A_ONES4:A_ONES4 + C]
            UUc = ta_sb[:, A_UUC:A_UUC + O]
            UUs = ta_sb[:, A_UUS:A_UUS + O]
            bb = ta_sb[0:C, A_BB:A_BB + O]
            TWsc = tb_sb[:, B_TWSC:B_TWSC + 2 * P]
            xwrep = tb_sb[:, B_XWREP:B_XWREP + 2 * P]
            xwin = tb_sb[:, B_XWIN:B_XWIN + C]
            ut = tc_sb[:]
            CVi = td_sb[:, D_CVI:D_CVI + P]
            SVi = td_sb[:, D_SVI:D_SVI + P]
            MA = td_sb[:, D_MA:D_MA + 2 * M * C]
            MB = td_sb[:, D_MB:D_MB + 2 * M * C]
            Mx = td_sb[0:C, D_MX:D_MX + M * C]

            # ---- carry-in over earlier blocks:  PP = [cosJ|sinJ]^T @ xm ----
            PP = ps.tile([P, 2 * M * C], F32)
            Px = ps.tile([C, M * C], F32)
            nc.tensor.matmul(PP[:, 0:M * C], cosJb, xm, start=True, stop=True)
            nc.tensor.matmul(PP[:, M * C:2 * M * C], sinJb, xm, start=True, stop=True)
            nc.tensor.matmul(Px[:], ones4, xm, start=True, stop=True)

            # ---- windowed products R[jj, (t,c,h)] = TW * xwin ----
            R = sb.tile([P, 2 * P], BF16)
            nc.vector.tensor_mul(R[:], TWsc, xwrep)

            # ---- carry masked reductions (vector STT with accum; gpsimd
            # cannot read PSUM) ----
            scrA = sb.tile([P, 2 * M * C], F32)
            scrB = sb.tile([P, 2 * M * C], F32)
            scrX = sb.tile([C, M * C], F32)
            col_s = sb.tile([P, 1], F32)
            col_c = sb.tile([P, 1], F32)
            pcx = sb.tile([C, 1], F32)
            nc.vector.scalar_tensor_tensor(
                scrA[:], PP[:], 1.0, MA, Mult, Mult, accum_out=col_s[:])
            nc.vector.scalar_tensor_tensor(
                scrB[:], PP[:], 1.0, MB, Mult, Mult, accum_out=col_c[:])
            nc.vector.scalar_tensor_tensor(
                scrX[:], Px[:], 1.0, Mx, Mult, Mult, accum_out=pcx[:])

            # ---- windowed causal prefix sums via triangular matmul ----
            pwS = ps.tile([P, P], F32)
            pwC = ps.tile([P, P], F32)
            pwx = ps.tile([C, P], F32)
            nc.tensor.matmul(pwS[:], R[:, 0:P], ut, start=True, stop=True)
            nc.tensor.matmul(pwC[:], R[:, P:2 * P], ut, start=True, stop=True)
            nc.tensor.matmul(pwx[:], xwin, ut, start=True, stop=True)

            # ---- S = window prefix + carry (bias adds, spread engines) ----
            S_s = sb.tile([P, P], BF16)
            S_c = sb.tile([P, P], BF16)
            Sx = sb.tile([C, P], BF16)
            nc.scalar.activation(S_s[:], pwS[:], Ident, bias=col_s[:])
            nc.vector.tensor_scalar_add(S_c[:], pwC[:], col_c[:])
            nc.scalar.activation(Sx[:], pwx[:], Ident, bias=pcx[:])

            # ---- Z1 = CVi*S_s - SVi*S_c (vector);  Z2 = CVi*S_c + SVi*S_s ----
            t1 = sb.tile([P, P], BF16)
            t2 = sb.tile([P, P], BF16)
            t3 = sb.tile([P, P], BF16)
            t4 = sb.tile([P, P], BF16)
            Z1 = sb.tile([P, P], BF16)
            Z2 = sb.tile([P, P], BF16)
            nc.vector.tensor_mul(t1[:], CVi, S_s[:])
            nc.vector.tensor_mul(t2[:], SVi, S_c[:])
            nc.vector.tensor_sub(Z1[:], t1[:], t2[:])
            nc.gpsimd.tensor_mul(t3[:], CVi, S_c[:])
            nc.gpsimd.tensor_mul(t4[:], SVi, S_s[:])
            nc.gpsimd.tensor_add(Z2[:], t3[:], t4[:])

            # ---- final contraction over p=(c,h), plus b2 term ----
            yps = ps.tile([O, P], F32)
            nc.tensor.matmul(yps[:], UUc, Z1[:], start=True, stop=False)
            nc.tensor.matmul(yps[:], UUs, Z2[:], start=False, stop=False)
            nc.tensor.matmul(yps[:], bb, Sx[:], start=False, stop=True)
            ysb = sb.tile([O, P], F32)
            nc.vector.tensor_copy(ysb[:], yps[:])
            nc.sync.dma_start(out=y[:], in_=ysb[:])
    nc.finalize()
    return nc


def _host_inputs(x, w1, b1, w2, b2):
    """Per-core input maps.  Host does layout, masking, replication, and
    weight-derived (x-independent) trig tables; all x-dependent math runs
    on device."""
    x = np.asarray(x, np.float64)
    w1 = np.asarray(w1, np.float64)
    b1 = np.asarray(b1, np.float64)
    w2 = np.asarray(w2, np.float64)[0]
    b2 = float(np.asarray(b2).reshape(-1)[0])

    A = w1[:, 0] / T                                   # [H]
    jj = np.arange(P)
    cJ = np.cos(A[None, :] * jj[:, None])              # [128, 32]
    sJ = np.sin(A[None, :] * jj[:, None])
    sB = np.sin(A[None, :] * P * np.arange(M)[:, None])  # [8, 32]
    cB = np.cos(A[None, :] * P * np.arange(M)[:, None])
    Ms = np.zeros((C, H, M, C))
    Mc = np.zeros((C, H, M, C))
    for c in range(C):
        Ms[c, :, :, c] = (w2[None, :] * sB).T
        Mc[c, :, :, c] = (w2[None, :] * cB).T
    Ms = Ms.reshape(P, M * C)
    Mc = Mc.reshape(P, M * C)
    Mx = np.zeros((C, M, C))
    for c in range(C):
        Mx[c, :, c] = 1.0
    Mx = Mx.reshape(C, M * C)
    off = (np.arange(C)[None, :, None] * w1[:, 1]
           + np.arange(O)[:, None, None] * w1[:, 2] + b1)   # [O, C, H]
    UUc = np.cos(off).transpose(1, 2, 0).reshape(P, O)
    UUs = np.sin(off).transpose(1, 2, 0).reshape(P, O)
    Ap = np.tile(A, C)                                  # [(c,h)]

    ta_base = np.zeros((P, NA), np.float64)
    ta_base[:, A_COSJ:A_COSJ + P] = np.tile(cJ, (1, C))
    ta_base[:, A_SINJ:A_SINJ + P] = np.tile(sJ, (1, C))
    ta_base[:, A_ONES4:A_ONES4 + C] = 1.0
    ta_base[:, A_UUC:A_UUC + O] = UUc
    ta_base[:, A_UUS:A_UUS + O] = UUs
    ta_base[0:C, A_BB:A_BB + O] = b2

    td_base = np.zeros((P, ND), np.float64)
    td_base[:, D_MA:D_MA + M * C] = Ms
    td_base[:, D_MA + M * C:D_MA + 2 * M * C] = Mc
    td_base[:, D_MB:D_MB + M * C] = Mc
    td_base[:, D_MB + M * C:D_MB + 2 * M * C] = -Ms
    td_base[0:C, D_MX:D_MX + M * C] = Mx

    ut = np.triu(np.ones((P, P))).astype(BF)

    in_maps = []
    for m in range(M):
        iabs = P * m + jj
        ta = ta_base.copy()
        xmask = x.copy()
        xmask[P * m:] = 0.0
        ta[:, A_XM:A_XM + M * C] = (
            xmask.reshape(M, P, C).transpose(1, 0, 2).reshape(P, M * C))

        tb = np.zeros((P, NB), np.float64)
        sin_i = np.sin(A[None, :] * iabs[:, None])      # [128, 32]
        cos_i = np.cos(A[None, :] * iabs[:, None])
        tb[:, B_TWSC:B_TWSC + P] = np.tile(w2[None, :] * sin_i, (1, C))
        tb[:, B_TWSC + P:B_TWSC + 2 * P] = np.tile(w2[None, :] * cos_i, (1, C))
        xw = x[P * m:P * m + P]
        tb[:, B_XWREP:B_XWREP + 2 * P] = np.tile(np.repeat(xw, H, axis=1), (1, 2))
        tb[:, B_XWIN:B_XWIN + C] = xw

        td = td_base.copy()
        td[:, D_CVI:D_CVI + P] = np.cos(Ap[:, None] * iabs[None, :])
        td[:, D_SVI:D_SVI + P] = np.sin(Ap[:, None] * iabs[None, :])

        in_maps.append({
            "ta": ta.astype(BF),
            "tb": tb.astype(BF),
            "tc": ut,
            "td": td.astype(BF),
        })
    return in_maps


def kernel(x, t, w1, b1, w2, b2, out_channels):
    if "nc" not in _nc_cache:
        _nc_cache["nc"] = _build_nc()
    nc = _nc_cache["nc"]
    in_maps = _host_inputs(x, w1, b1, w2, b2)
    res = run_bass_kernel_spmd(nc, in_maps, core_ids=list(range(M)))
    y = np.empty((T, O), np.float32)
    for m in range(M):
        ym = np.asarray(res.results[m]["y"]).reshape(O, P)
        y[P * m:P * (m + 1), :] = ym.T
    return y
